# revision 5
# baseline (speedup 1.0000x reference)
"""Trainium2 Bass kernel for nn_BasicCSRNN (bottom-up tree RNN), v2.

Strategy: shard the 16384 level-0 SUBTREES across 8 cores (balanced by
active-node count); each core keeps the full H=256 hidden dim, so every
scatter-add is core-local and each 16KB sel weight-load amortizes over
256 rhs columns. Per level, active srcs are sorted by parent slot and
placed so src block s only feeds dst windows {s, s+1} (plus a few
compile-time C-entries where children spill into block t+1) - a uniform
program across cores (SPMD), per-core content. PSUM seeding uses a
per-window [17,128] fp8 Cnt matrix (ones row for cb + per-etype counts
of inactive children) against a shared [17,256] rhs = [cb; tanh(cb)*We],
replacing fat A_const streams. tanh on ACT reads PSUM; DVE multiplies by
the fp8 scale stream; level-0 m reduces to the root partial via a ones
matmul. Host sums the 8 root partials + the root constant and applies
the final tanh.
"""
import os
import sys

sys.path.insert(0, "/opt/trn_rl_repo")
import numpy as np

D, W = 16, 16384
N = 1 + (D - 1) * W
H, I, E = 256, 256, 16
NCORES = 8
NLVL = D - 2  # 14 computed levels (0..13); level 14 = leaves

_cache = {}
LAST_EXEC_NS = None


def _install_profhook():
    """Register the NTFF profile hook so trace=True works under axon."""
    import types
    try:
        from antenv import axon_hooks  # noqa: F401
        return
    except ImportError:
        pass
    import antenv
    mod = types.ModuleType("antenv.axon_hooks")
    _hook = [None]
    mod.set_axon_ntff_profile_hook = lambda h: _hook.__setitem__(0, h)
    mod.get_axon_ntff_profile_hook = lambda: _hook[0]
    sys.modules["antenv.axon_hooks"] = mod
    antenv.axon_hooks = mod
    from trn_agent_boot.trn_boot import _ntff_profile_via_ctypes
    mod.set_axon_ntff_profile_hook(
        _ntff_profile_via_ctypes("/opt/axon/libaxon_pjrt.so"))
    import concourse.bass_utils as bu
    bu.upload_artifacts = lambda tmpdir: "local://" + str(tmpdir)


def _build_structure(parent, etype, levels, is_rel):
    """Host-side layout build (call-independent, cached).

    Returns dict with global NB per level, per-core slot_nodes, sel/cnt
    byte streams, and the C-entry window sets per level.
    """
    import ml_dtypes
    f8 = ml_dtypes.float8_e4m3
    lv = [np.asarray(levels[d], np.int64) for d in range(D - 1)]
    cnt = np.zeros(N, np.int64)
    for d in range(D - 1):
        np.add.at(cnt, parent[lv[d]], 1)
    alive = np.zeros(N, bool)
    alive[0] = True
    for d in range(D - 1):
        p = parent[lv[d]]
        alive[lv[d]] = alive[p] & ~is_rel[p]
    active = alive & ~is_rel & (cnt > 0)
    active[0] = False

    # subtree balance: assign each level-0 subtree to a core
    anc = np.zeros(N, np.int64)
    anc[lv[0]] = lv[0]
    for d in range(1, D - 1):
        anc[lv[d]] = anc[parent[lv[d]]]
    sub_act = np.zeros(N, np.int64)
    np.add.at(sub_act, anc[1:], active[1:].astype(np.int64))
    roots = lv[0]
    wts = sub_act[roots]
    order = np.argsort(-wts, kind="stable")
    load = np.zeros(NCORES, np.int64)
    core_of_root = np.zeros(len(roots), np.int64)
    for i in order:
        c = int(np.argmin(load))
        core_of_root[i] = c
        load[c] += wts[i]
    core_of = np.full(N, -1, np.int64)
    core_of[roots] = core_of_root
    for d in range(1, D - 1):
        core_of[lv[d]] = core_of[anc[lv[d]]]

    # inactive-alive children contribute tanh(cb)*We[etype]; count per
    # (parent, etype) for the Cnt seed matmuls (root excluded, host-side)
    ina = alive & ~active
    pe_ = parent[1:][ina[1:]]
    ee_ = etype[1:][ina[1:]]
    m_ = pe_ != 0
    cnt16 = np.zeros((N, E), np.int16)
    np.add.at(cnt16, (pe_[m_], ee_[m_]), 1)
    assert cnt16.max() < 16, "per-etype count not fp8-exact"

    # pass 1: per-core placement, collect per-core layouts + spill sets
    per_core = []
    spills = {}  # level d (src side, 1..13) -> set of dst windows
    for c in range(NCORES):
        slotpos = np.full(N, -1, np.int64)
        act0 = lv[0][(active[lv[0]]) & (core_of[lv[0]] == c)]
        sn0 = list(act0)
        if len(sn0) % 128:
            sn0 += [-1] * (128 - len(sn0) % 128)
        sn0 = np.array(sn0, np.int64)
        slotpos[sn0[sn0 >= 0]] = np.nonzero(sn0 >= 0)[0]
        slot_nodes = [sn0]
        nb_prev = len(sn0) // 128
        for d in range(1, NLVL):
            nodes = lv[d][(active[lv[d]]) & (core_of[lv[d]] == c)]
            dp = slotpos[parent[nodes]]
            o = np.argsort(dp, kind="stable")
            nodes, dp = nodes[o], dp[o]
            NW = nb_prev
            cur = 0
            sn = np.full(NW * 128 + 256, -1, np.int64)
            i = 0
            for t in range(NW):
                j = i
                while j < len(dp) and dp[j] < 128 * (t + 1):
                    j += 1
                ct = j - i
                start = max(cur, 128 * (t - 1)) if t > 0 else 0
                end = start + ct
                if end > 128 * (t + 1):
                    assert end <= 128 * (t + 2), "2-block spill"
                    spills.setdefault(d, set()).add(t)
                sn[start:end] = nodes[i:j]
                cur = end
                i = j
            nb = (cur + 127) // 128
            sn = sn[:nb * 128]
            slotpos[sn[sn >= 0]] = np.nonzero(sn >= 0)[0]
            slot_nodes.append(sn)
            nb_prev = nb
        per_core.append((slot_nodes, slotpos))

    NB = [max(len(per_core[c][0][d]) // 128 for c in range(NCORES))
          for d in range(NLVL)]
    S = {d: sorted(spills.get(d + 1, ())) for d in range(NLVL - 1)}
    # S[d] = dst windows of level d needing a C entry (srcs at level d+1)

    # pass 2: build per-core sel/cnt streams at the global NB sizes
    cores = []
    for c in range(NCORES):
        slot_nodes, slotpos = per_core[c]
        sn_pad = []
        for d in range(NLVL):
            sn = np.full(NB[d] * 128, -1, np.int64)
            sn[:len(slot_nodes[d])] = slot_nodes[d]
            sn_pad.append(sn)
        # sel stream: [B_{w-1} | A_w] DoubleRow pairs, cols 2w*128.
        # C-entry sel blocks live in the CNT stream (paired with seeds).
        sels = {}
        cnts = {}
        for d in range(NLVL):
            nC = len(S[d]) if d < NLVL - 1 else 0
            cs = np.zeros((128, (NB[d] + nC + 1) * 128), f8)
            cs[0:3, :NB[d] * 128] = 1.0
            sn = sn_pad[d]
            real = sn >= 0
            cs[3:19, np.nonzero(real)[0]] = cnt16[sn[real]].T.astype(f8)
            cs[19:35, np.nonzero(real)[0]] = cnt16[sn[real]].T.astype(f8)
            cnts[d] = cs
        for d in range(NLVL - 1):  # dst level d, srcs level d+1
            sel = np.zeros((128, 2 * NB[d] * 128), f8)
            sn = sn_pad[d + 1]
            real = np.nonzero(sn >= 0)[0]
            dpv = slotpos[parent[sn[real]]]
            sblk, k = real // 128, real % 128
            t, j = dpv // 128, dpv % 128
            cidx = {t_: i for i, t_ in enumerate(S[d])}
            for n_ in range(len(real)):
                s_, k_, t_, j_ = sblk[n_], k[n_], t[n_], j[n_]
                if t_ == s_:
                    sel[k_, (2 * t_ + 1) * 128 + j_] = 1.0
                elif t_ == s_ - 1:
                    # C entry: block t+1 -> window t, in cnt stream tail
                    cnts[d][k_, (NB[d] + cidx[t_]) * 128 + j_] = 1.0
                elif t_ == s_ + 1:
                    sel[k_, (2 * t_) * 128 + j_] = 1.0
                else:
                    raise AssertionError("zone violation")
            sels[d] = sel
        combos = {}
        for d in range(NLVL):
            parts = [cnts[d]]
            if d < NLVL - 1:
                parts.append(sels[d])
            combos[d] = np.ascontiguousarray(np.concatenate(parts, 1))
        cores.append({"sn": sn_pad, "combo": combos})

    return {"NB": NB, "S": S, "cores": cores, "active": active,
            "alive": alive}


def _compile(NB, S):
    import concourse.bacc as bacc
    import concourse.bass as bass
    import concourse.mybir as mybir
    import concourse.tile as tile

    f32 = mybir.dt.float32
    f16 = mybir.dt.float16
    f8 = mybir.dt.float8e4
    DR = mybir.MatmulPerfMode.DoubleRow

    nc = bacc.Bacc("TRN2", target_bir_lowering=False, debug=False,
                   num_devices=NCORES)
    seed_in = nc.dram_tensor("seedf8", [128, 2 * H], f8,
                             kind="ExternalInput")  # [seed | zeros]
    ones_in = nc.dram_tensor("ones", [128, 1], f16, kind="ExternalInput")
    combo_in = {}
    sc0_in = None
    for d in range(NLVL):
        nC = len(S[d]) if d < NLVL - 1 else 0
        ncol = (NB[d] + nC + 1) * 128
        if d < NLVL - 1:
            ncol += 2 * NB[d] * 128
        if d > 0:
            ncol += NB[d] * H
        combo_in[d] = nc.dram_tensor(f"combo{d}", [128, ncol], f8,
                                     kind="ExternalInput")
    sc0_in = nc.dram_tensor("sc0", [128, NB[0] * H], f16,
                            kind="ExternalInput")
    root_out = nc.dram_tensor("root", [1, H], f32, kind="ExternalOutput")

    def ap3(t, off, kstride, kn, inner):
        base = t[:]
        return bass.AP(base.tensor, base.offset + off,
                       [[base.ap[0][0], 128], [kstride, kn], [1, inner]])

    def ap3s(base, off, kstride, kn, inner):
        return bass.AP(base.tensor, base.offset + off,
                       [[base.ap[0][0], 128], [kstride, kn], [1, inner]])

    with tile.TileContext(nc) as tc:
        with tc.tile_pool(name="const", bufs=1) as cpool, \
             tc.tile_pool(name="stream", bufs=1) as spool, \
             tc.tile_pool(name="m", bufs=1) as mpool, \
             tc.tile_pool(name="hh", bufs=2) as hpool, \
             tc.tile_pool(name="psum", bufs=3, space="PSUM") as pp:
            ones_t = cpool.tile([128, 1], f16, tag="ones")
            nc.sync.dma_start(out=ones_t[:], in_=ones_in[:])

            # m tiles: level 0 plain f16; levels >=1 fp8 with a prepended
            # seed block (tile block 0 = seed rhs for the consumer level's
            # seed DR matmul). m14 = dummy [seed | zeros] for level 13.
            m_t = {}
            m_t[0] = mpool.tile([128, NB[0] * H], f16, tag="m0", name="m0")
            for d in range(1, NLVL):
                m_t[d] = mpool.tile([128, (NB[d] + 1) * H], f8,
                                    tag=f"m{d}", name=f"m{d}")
            m_t[NLVL] = mpool.tile([128, 2 * H], f8, tag="m14", name="m14")

            # streams: one combo DMA per level, issued deepest-first with
            # that level's m seed block so compute can start immediately
            combo_t, cnt_t, sel_t, sc_t = {}, {}, {}, {}
            qi = 0
            for d in range(NLVL - 1, -1, -1):
                nC = len(S[d]) if d < NLVL - 1 else 0
                ncol = (NB[d] + nC + 1) * 128
                sel_off = ncol
                if d < NLVL - 1:
                    ncol += 2 * NB[d] * 128
                sc_off = ncol
                if d > 0:
                    ncol += NB[d] * H
                combo_t[d] = spool.tile([128, ncol], f8, tag=f"combo{d}",
                                        name=f"combo{d}")
                eng = nc.gpsimd if qi % 2 else nc.sync
                if d == NLVL - 1:
                    # critical path: cnt13 lands first, m14 in parallel
                    cut = (NB[d] + nC + 1) * 128
                    nc.sync.dma_start(out=combo_t[d][:, :cut],
                                      in_=combo_in[d][:, :cut])
                    nc.gpsimd.dma_start(out=m_t[NLVL][:], in_=seed_in[:])
                    nc.sync.dma_start(out=combo_t[d][:, cut:],
                                      in_=combo_in[d][:, cut:])
                else:
                    eng.dma_start(out=combo_t[d][:], in_=combo_in[d][:])
                if d > 0:
                    eng2 = nc.sync if qi % 2 else nc.gpsimd
                    eng2.dma_start(out=m_t[d][:, 0:H], in_=seed_in[:, 0:H])
                cnt_t[d] = combo_t[d]
                if d < NLVL - 1:
                    sel_t[d] = combo_t[d][:, sel_off:sc_off]
                if d > 0:
                    sc_t[d] = combo_t[d][:, sc_off:ncol]
                qi += 1
            sc_t[0] = spool.tile([128, NB[0] * H], f16, tag="sc0f",
                                 name="sc0f")
            nc.sync.dma_start(out=sc_t[0][:], in_=sc0_in[:])

            rps = pp.tile([128, H], f32, tag="rps", bufs=1, name="rps")

            # wavefront schedule across levels: deepest-first diagonal
            SKEW = 2.5
            items = []
            for d in range(NLVL):
                for p in range((NB[d] + 1) // 2):
                    items.append((p + SKEW * (NLVL - 1 - d), -d, p))
            items.sort()
            items = [(-negd, p) for _, negd, p in items]

            LAG = 3

            def emit_seeds(d, p, w0, nw, ps2):
                nC = len(S[d]) if d < NLVL - 1 else 0
                nbs = NB[d + 1] if d < NLVL - 1 else 1
                mp = m_t[d + 1] if d < NLVL - 1 else m_t[NLVL]
                for wi in range(nw):
                    w = w0 + wi
                    ps = ps2[:, wi * H:(wi + 1) * H]
                    # k1 is a zero lhsT block; rhs k1 re-reads the seed
                    # block (stride 0) so no not-yet-written m is touched
                    nc.tensor.matmul(
                        out=ps,
                        lhsT=ap3(cnt_t[d], w * 128,
                                 (NB[d] + nC - w) * 128, 2, 128),
                        rhs=ap3(mp, 0, 0, 2, H),
                        start=(wi == 0), stop=True,
                        skip_group_check=True, perf_mode=DR)

            def emit_scatters(d, p, w0, nw, ps2):
                nbs = NB[d + 1] if d < NLVL - 1 else 1
                for wi in range(nw):
                    w = w0 + wi
                    ps = ps2[:, wi * H:(wi + 1) * H]
                    if not (d < NLVL - 1 and w <= nbs):
                        continue
                    mp = m_t[d + 1]
                    has_c = w in S[d] and w + 1 < nbs
                    if w < nbs:
                        nc.tensor.matmul(
                            out=ps,
                            lhsT=ap3s(sel_t[d], 2 * w * 128, 128, 2, 128),
                            rhs=ap3(mp, w * H, H, 2, H),
                            start=False, stop=not has_c,
                            skip_group_check=True, perf_mode=DR)
                    else:  # w == nbs: only B exists, normal matmul
                        nc.tensor.matmul(
                            out=ps,
                            lhsT=sel_t[d][:, 2 * w * 128:(2 * w + 1) * 128],
                            rhs=mp[:, w * H:(w + 1) * H],
                            start=False, stop=not has_c,
                            skip_group_check=True)
                    if has_c:
                        ci = S[d].index(w)
                        nc.tensor.matmul(
                            out=ps,
                            lhsT=cnt_t[d][:, (NB[d] + ci) * 128:
                                          (NB[d] + ci + 1) * 128],
                            rhs=mp[:, (w + 2) * H:(w + 3) * H],
                            start=False, stop=True,
                            skip_group_check=True)

            def emit_tail(d, p, w0, nw, ps2, qn):
                hh = hpool.tile([128, 2 * H], f16, tag=f"hh{qn % 6}",
                                name="hh")
                nc.scalar.activation(
                    out=hh[:, :nw * H], in_=ps2[:, :nw * H],
                    func=mybir.ActivationFunctionType.Tanh)
                moff = 0 if d == 0 else H
                nc.vector.tensor_tensor(
                    out=m_t[d][:, moff + w0 * H:moff + (w0 + nw) * H],
                    in0=hh[:, :nw * H],
                    in1=sc_t[d][:, w0 * H:(w0 + nw) * H],
                    op=mybir.AluOpType.mult)
                if d == 0:
                    for wi in range(nw):
                        w = w0 + wi
                        nc.tensor.matmul(
                            out=rps[0:1, :], lhsT=ones_t[:],
                            rhs=m_t[0][:, w * H:(w + 1) * H],
                            start=(w == 0), stop=(w == NB[0] - 1),
                            skip_group_check=True)

            # software-pipelined emission: seeds run LAG items ahead
            state = []   # per item: (d, p, w0, nw, ps2)
            for qn, (d, p) in enumerate(items):
                w0 = 2 * p
                nw = min(2, NB[d] - w0)
                ps2 = pp.tile([128, 2 * H], f32, tag="ps", bufs=7,
                              name="ps2")
                state.append((d, p, w0, nw, ps2))
                emit_seeds(d, p, w0, nw, ps2)
                if qn >= LAG:
                    d2, p2, w02, nw2, ps22 = state[qn - LAG]
                    emit_scatters(d2, p2, w02, nw2, ps22)
                    emit_tail(d2, p2, w02, nw2, ps22, qn - LAG)
            for j in range(max(0, len(items) - LAG), len(items)):
                d2, p2, w02, nw2, ps22 = state[j]
                emit_scatters(d2, p2, w02, nw2, ps22)
                emit_tail(d2, p2, w02, nw2, ps22, j)

            rout = cpool.tile([1, H], f32, tag="ro")
            nc.vector.tensor_copy(out=rout[:], in_=rps[0:1, :])
            nc.sync.dma_start(out=root_out[:], in_=rout[:])

    nc.finalize()
    return nc


def kernel(embedding, Wx, We, b, parent, etype, levels, is_rel):
    import ml_dtypes
    from concourse.bass_utils import run_bass_kernel_spmd
    f8 = ml_dtypes.float8_e4m3

    embedding = np.asarray(embedding, np.float32)
    Wx = np.asarray(Wx, np.float32)
    We = np.asarray(We, np.float32)
    b = np.asarray(b, np.float32)
    parent = np.asarray(parent, np.int64)
    etype = np.asarray(etype, np.int64)
    levels_np = np.asarray(levels, np.int64)
    is_rel = np.asarray(is_rel, bool)

    import hashlib
    key = hashlib.sha1(parent.tobytes() + etype.tobytes()
                       + is_rel.tobytes() + levels_np.tobytes()).hexdigest()
    if key not in _cache:
        st = _build_structure(parent, etype, levels_np, is_rel)
        nc = _compile(st["NB"], st["S"])
        _cache[key] = (st, nc)
    st, nc = _cache[key]
    NB, S = st["NB"], st["S"]

    # ---- per-call numerics ----
    c = embedding @ Wx
    cb = (c + b[0]).astype(np.float32)
    tanhcb = np.tanh(cb)
    WeT = We[:, 0, :]                       # [E, H]

    def f8split(x, terms):
        """split x into `terms` fp8 rows summing to ~x"""
        out = []
        r = x.astype(np.float32)
        for _ in range(terms):
            q = r.astype(f8)
            out.append(q)
            r = r - q.astype(np.float32)
        return out

    tw = tanhcb[None, :] * WeT              # [E, H]
    seed = np.zeros((128, 2 * H), f8)
    cbs = f8split(cb, 3)
    for i in range(3):
        seed[i, :H] = cbs[i]
    tws = f8split(tw, 2)
    seed[3:19, :H] = tws[0]
    seed[19:35, :H] = tws[1]

    in_maps = []
    for core in range(NCORES):
        cs = st["cores"][core]
        m = {"seedf8": seed,
             "ones": np.ones((128, 1), np.float16)}
        for d in range(NLVL):
            sn = cs["sn"][d]
            real = sn >= 0
            e = etype[np.where(real, sn, 0)]
            sc = WeT[e] * real[:, None]     # [slots, H]
            nbd = NB[d]
            sc = sc.reshape(nbd, 128, H).transpose(1, 0, 2).reshape(
                128, nbd * H)
            if d == 0:
                m["combo0"] = cs["combo"][0]
                m["sc0"] = sc.astype(np.float16)
            else:
                m[f"combo{d}"] = np.ascontiguousarray(np.concatenate(
                    [cs["combo"][d], sc.astype(f8)], 1))
        in_maps.append(m)

    trace = bool(os.environ.get("CSRNN_TRACE"))
    kw = {}
    if trace:
        import tempfile
        _install_profhook()
        kw = {"trace": True, "tmpdir": tempfile.mkdtemp(prefix="csrnn_")}
    res = run_bass_kernel_spmd(nc, in_maps, list(range(NCORES)), **kw)
    global LAST_EXEC_NS
    LAST_EXEC_NS = res.exec_time_ns

    # root constant: inactive-alive level-0 nodes contribute tanh(cb)*We[e]
    lv0 = levels_np[0]
    ina0 = lv0[~st["active"][lv0]]
    hist = np.bincount(etype[ina0], minlength=E).astype(np.float64)
    root_const = tanhcb * (hist @ WeT)

    partials = np.stack([res.results[core]["root"][0]
                         for core in range(NCORES)])
    root_hidden = partials.sum(0) + root_const
    if is_rel[0]:
        root_hidden = np.zeros_like(root_hidden)
    out = np.tanh(c + root_hidden + b[0])
    return out[None, :].astype(np.float32)
